# revision 1
# baseline (speedup 1.0000x reference)
"""Trainium2 Bass kernel for nn_CADense (context-adaptive low-rank dense layer).

Computes, for the full batch:
    s_mod = s + context @ w          # [B, R]
    low   = (data @ u) * s_mod       # [B, R]
    out   = relu(low @ v.T + 2*bias) # [B, UNITS]

Sharding: data-parallel over batch across 8 NeuronCores; u/s/v/w/bias
replicated. Each core runs the same Bass program on its 1024-row shard.

The PE contracts over the partition dim, so the big operands are marshaled
host-side into contraction-major layouts (data.T, context.T, v.T) when the
shards are built — on-chip PE transposes would otherwise dominate the
kernel. All matmuls run as float32r (full-rate fp32 streaming mode).

Compute is done in the "transposed" domain per rank-chunk:
    lowT[r, b] = (u.T @ data.T)[r, b] * (s[r] + (w.T @ ctx.T)[r, b])
with the s-add fused into the scalar-engine PSUM evacuation. The final
matmul returns to natural [b, units] layout; the 2*bias add is folded in
as a K=1 rank-1 matmul into the same PSUM accumulation group and ReLU
evacuation of the output PSUM groups alternates between the scalar and
vector engines so neither gates PSUM recycling.

Schedule notes:
- Input DMAs are spread across both HWDGE queues (sync: data tiles,
  scalar: weights/context) and output stores go through the gpsimd SWDGE
  queue — three independent descriptor rings so transfers overlap and
  the HBM link stays saturated.
- The two 512-row batch tiles are software-pipelined; PE emission
  interleaves batch-tile 1's rank stage with batch-tile 0's output stage
  and the (DMA-independent) context matmuls fill data-DMA wait bubbles,
  so the PE never idles long enough for the HAM clock gate to
  re-throttle.
- A short burst of bf16 dummy matmuls on garbage SBUF pre-warms the HAM
  clock gate while the first DMAs stream in.
"""

import os
import sys
from contextlib import ExitStack

import numpy as np


def _ensure_concourse():
    try:
        import concourse  # noqa: F401
    except ImportError:
        for p in ("/opt/trn_rl_repo", "/root/.axon_site/_ro/trn_rl_repo"):
            if os.path.isdir(p) and p not in sys.path:
                sys.path.insert(0, p)


_ensure_concourse()

import concourse.tile as tile  # noqa: E402
from concourse import bacc, mybir  # noqa: E402
from concourse.bass_utils import run_bass_kernel_spmd  # noqa: E402

NCORES = 8
B, N_IN, UNITS, RANK, CCTX = 8192, 2048, 2048, 256, 512
NB = B // NCORES  # batch rows per core
P = 128
BT = 512  # batch tile (free dim of T-domain matmuls)
NBT = NB // BT  # batch tiles per core
KC = N_IN // P  # 16 contraction chunks for data @ u
CC = CCTX // P  # 4 contraction chunks for context @ w
RC = RANK // P  # 2 rank chunks
MS = 512  # output units slice width
NMS = UNITS // MS  # 4 unit slices
N_WARMUP_MM = 14

F32 = mybir.dt.float32
F32R = mybir.dt.float32r
BF16 = mybir.dt.bfloat16


def _emit(nc, tc, ctx):
    # Host-marshaled transposed layouts: dataT = data.T, ctxT = context.T,
    # vT = v.T (built per-shard in kernel()).
    d_dataT = nc.dram_tensor("dataT", [N_IN, NB], F32R, kind="ExternalInput")
    d_ctxT = nc.dram_tensor("ctxT", [CCTX, NB], F32R, kind="ExternalInput")
    d_u = nc.dram_tensor("u", [N_IN, RANK], F32R, kind="ExternalInput")
    d_s = nc.dram_tensor("s", [RANK], F32, kind="ExternalInput")
    d_vT = nc.dram_tensor("vT", [RANK, UNITS], F32R, kind="ExternalInput")
    d_w = nc.dram_tensor("w", [CCTX, RANK], F32R, kind="ExternalInput")
    d_bias = nc.dram_tensor("bias", [UNITS], F32R, kind="ExternalInput")
    d_out = nc.dram_tensor("out", [NB, UNITS], F32, kind="ExternalOutput")

    ap_dataT = d_dataT.ap().rearrange("(q j p) b -> p q j b", p=P, j=4)
    ap_ctxT = d_ctxT.ap().rearrange("(cc p) b -> p cc b", p=P)
    ap_u = d_u.ap().rearrange("(uq j p) r -> p uq j r", p=P, j=4)
    ap_vT = d_vT.ap().rearrange("(rc p) m -> p rc m", p=P)

    singles = ctx.enter_context(tc.tile_pool(name="singles", bufs=1))
    du_psum = ctx.enter_context(tc.tile_pool(name="du_psum", bufs=2, space="PSUM"))
    s_psum = ctx.enter_context(tc.tile_pool(name="s_psum", bufs=2, space="PSUM"))
    o_psum = ctx.enter_context(tc.tile_pool(name="o_psum", bufs=4, space="PSUM"))
    dTpool = ctx.enter_context(tc.tile_pool(name="dataT", bufs=1))
    cTpool = ctx.enter_context(tc.tile_pool(name="ctxT", bufs=2))
    lowpool = ctx.enter_context(tc.tile_pool(name="lowT", bufs=2))
    smodpool = ctx.enter_context(tc.tile_pool(name="smod", bufs=4))
    opool = ctx.enter_context(tc.tile_pool(name="outsb", bufs=3))

    # HAM warm-up fodder: garbage bf16 matmuls while the first loads stream.
    wu_a = singles.tile([P, P], BF16)
    nc.vector.memset(wu_a[:], 1.0)
    wu_b = singles.tile([P, MS], BF16)
    nc.vector.memset(wu_b[:], 1.0)

    # ---- input DMA queue (sync ring), in first-use order ---------------
    # A single HWDGE ring sustains ~360 GB/s for 0.5-1 MiB transfers; the
    # order below is by first consumption: u/dataT0 pairs (rank stage 0),
    # w/ctx (s_mod), vT (output stage 0), then dataT1 (rank stage 1).
    # Output stores ride the gpsimd SWDGE ring so they never queue ahead
    # of loads.
    dataT_t = {0: []}
    dq = {}
    for q4 in range(4):
        dq[(0, q4)] = dTpool.tile(
            [P, 4, BT], F32R, tag=f"dataT0q{q4}", name=f"dataT0q{q4}"
        )
    dataT_t[0] = [dq[(0, q4)][:, j] for q4 in range(4) for j in range(4)]
    # batch-tile 1 is loaded as two 256-row halves so the final output
    # stage is gated by only half the remaining data.
    dqh = {}
    dataT_h = {}
    for h in range(2):
        for q4 in range(4):
            dqh[(h, q4)] = dTpool.tile(
                [P, 4, 256], F32R, tag=f"dataT1h{h}q{q4}", name=f"dataT1h{h}q{q4}"
            )
        dataT_h[h] = [dqh[(h, q4)][:, j] for q4 in range(4) for j in range(4)]
    u_t = []  # u_t[uq] = [P, 4, RANK] tile; chunk kc = u_t[kc//4][:, kc%4]
    for uq in range(4):
        ut = singles.tile([P, 4, RANK], F32R, name=f"uq{uq}")
        u_t.append(ut)

    nc.sync.dma_start(out=u_t[0][:], in_=ap_u[:, 0])
    nc.sync.dma_start(out=dq[(0, 0)][:], in_=ap_dataT[:, 0, :, 0:BT])
    w_sb = singles.tile([P, CC, RANK], F32R)
    nc.sync.dma_start(
        out=w_sb[:], in_=d_w.ap().rearrange("(cc p) r -> p cc r", p=P)
    )
    ctxT_t = {}
    ctxT_t[0] = cTpool.tile([P, CC, BT], F32R, tag="ctxT", name="ctxT0")
    nc.sync.dma_start(out=ctxT_t[0][:], in_=ap_ctxT[:, :, 0:BT])
    s_sb = singles.tile([P, RC], F32)
    nc.sync.dma_start(out=s_sb[:], in_=d_s.ap().rearrange("(rc p) -> p rc", p=P))
    bias2 = singles.tile([1, UNITS], F32R)
    nc.sync.dma_start(out=bias2[:], in_=d_bias.ap().rearrange("(a m) -> a m", a=1))
    for uq in (1, 2, 3):
        nc.sync.dma_start(out=u_t[uq][:], in_=ap_u[:, uq])
        nc.sync.dma_start(out=dq[(0, uq)][:], in_=ap_dataT[:, uq, :, 0:BT])
    vT_sb = singles.tile([P, RC, UNITS], F32R)
    nc.sync.dma_start(out=vT_sb[:, 0], in_=ap_vT[:, 0])
    nc.sync.dma_start(out=vT_sb[:, 1], in_=ap_vT[:, 1])
    ctxT_t[1] = cTpool.tile([P, CC, BT], F32R, tag="ctxT", name="ctxT1")
    nc.sync.dma_start(out=ctxT_t[1][:], in_=ap_ctxT[:, :, BT:])
    for h in range(2):
        for q4 in range(4):
            nc.sync.dma_start(
                out=dqh[(h, q4)][:],
                in_=ap_dataT[:, q4, :, BT + h * 256 : BT + (h + 1) * 256],
            )

    ones_f = singles.tile([1, P], F32)
    nc.vector.memset(ones_f[:], 2.0)
    ones = singles.tile([1, P], F32R)
    nc.vector.tensor_copy(out=ones[:], in_=ones_f[:])

    # ---- HAM warm-up ---------------------------------------------------
    wu_ps = o_psum.tile([P, MS], F32, tag="po", name="wu_ps")
    for _ in range(N_WARMUP_MM):
        nc.tensor.matmul(wu_ps[:], lhsT=wu_a[:], rhs=wu_b[:], start=True, stop=True)

    # ---- compute stages ------------------------------------------------
    lowT_t = {}
    pd_t = {}
    smod_t = {}

    def emit_warm_keepers(n):
        """No-dep bf16 matmuls that keep the HAM activity monitor above
        its throttle threshold while real matmuls are DMA-paced."""
        for _ in range(n):
            nc.tensor.matmul(
                wu_ps[:], lhsT=wu_a[:], rhs=wu_b[:], start=True, stop=True
            )

    def emit_rank_mms(key, chunks, width, kc_lo, kc_hi, keepers=False):
        """mm1T k-chunks [kc_lo, kc_hi) for both rank chunks."""
        if kc_lo == 0:
            pd_t[key] = [
                du_psum.tile([P, width], F32, tag="pd", name="pd")
                for _ in range(RC)
            ]
        for kc in range(kc_lo, kc_hi):
            for rc in range(RC):
                nc.tensor.matmul(
                    pd_t[key][rc][:],
                    lhsT=u_t[kc // 4][:, kc % 4, rc * P : (rc + 1) * P],
                    rhs=chunks[kc],
                    start=(kc == 0),
                    stop=(kc == KC - 1),
                )
            if keepers and kc % 2 == 1:
                emit_warm_keepers(2)

    def emit_smod(bt):
        """ctx @ w matmuls + s-add; independent of the data stream."""
        smod_t[bt] = []
        for rc in range(RC):
            ps = s_psum.tile([P, BT], F32, tag="ps", name="ps")
            for cc in range(CC):
                nc.tensor.matmul(
                    ps[:],
                    lhsT=w_sb[:, cc, rc * P : (rc + 1) * P],
                    rhs=ctxT_t[bt][:, cc, :],
                    start=(cc == 0),
                    stop=(cc == CC - 1),
                )
            smod = smodpool.tile([P, BT], F32, tag="smod", name="smod")
            nc.scalar.add(smod[:], ps[:], add=s_sb[:, rc : rc + 1])
            smod_t[bt].append(smod)

    def emit_mul(key, bt, width, off=0):
        """lowT = pd * smod on the vector engine."""
        lowT_t[key] = lowpool.tile(
            [P, RC, width], F32R, tag=f"lowT{width}", name="lowT"
        )
        for rc in range(RC):
            nc.vector.tensor_mul(
                out=lowT_t[key][:, rc, :],
                in0=pd_t[key][rc][:],
                in1=smod_t[bt][rc][:, off : off + width],
            )

    def emit_out_stage(key, row0, bc, fine_stores=False, store_engine=None, split_store=False):
        """out[b, :] = relu(low @ v.T + 2*bias) for one 128-row chunk.

        All four 512-wide PSUM groups stay open at once and the matmuls
        are ordered rc-major so consecutive matmuls reuse the same
        stationary operand; ReLU evacuation alternates between the
        scalar and vector engines.
        """
        lowT = lowT_t[key]
        osb = opool.tile([P, UNITS], F32, tag="osb", name="osb")
        pos = [o_psum.tile([P, MS], F32, tag="po", name="po") for _ in range(NMS)]
        for rc in range(RC):
            for ms in range(NMS):
                nc.tensor.matmul(
                    pos[ms][:],
                    lhsT=lowT[:, rc, bc * P : (bc + 1) * P],
                    rhs=vT_sb[:, rc, ms * MS : (ms + 1) * MS],
                    start=(rc == 0),
                    stop=False,
                )
        for ms in range(NMS):
            nc.tensor.matmul(
                pos[ms][:],
                lhsT=ones[:],
                rhs=bias2[:, ms * MS : (ms + 1) * MS],
                start=False,
                stop=True,
            )
        rows = slice(row0 + bc * P, row0 + (bc + 1) * P)
        eng = store_engine if store_engine is not None else nc.gpsimd
        for ms in range(NMS):
            sl = slice(ms * MS, (ms + 1) * MS)
            if ms % 2 == 0:
                nc.scalar.activation(
                    osb[:, sl], pos[ms][:], mybir.ActivationFunctionType.Relu
                )
            else:
                nc.vector.tensor_relu(out=osb[:, sl], in_=pos[ms][:])
            if fine_stores:
                seng = nc.sync if ms % 2 == 0 else nc.gpsimd
                seng.dma_start(out=d_out.ap()[rows, sl], in_=osb[:, sl])
        if not fine_stores:
            if split_store:
                h = UNITS // 2
                nc.gpsimd.dma_start(out=d_out.ap()[rows, :h], in_=osb[:, :h])
                nc.sync.dma_start(out=d_out.ap()[rows, h:], in_=osb[:, h:])
            else:
                eng.dma_start(out=d_out.ap()[rows, :], in_=osb[:])

    # Software pipeline, PE emission ordered to match DMA arrival order.
    emit_rank_mms(0, dataT_t[0], BT, 0, 4, keepers=True)
    emit_rank_mms(0, dataT_t[0], BT, 4, 8, keepers=True)
    emit_smod(0)
    emit_rank_mms(0, dataT_t[0], BT, 8, 12, keepers=True)
    emit_rank_mms(0, dataT_t[0], BT, 12, 16, keepers=True)
    emit_mul(0, 0, BT)
    emit_out_stage(0, 0, 0)
    emit_out_stage(0, 0, 1)
    emit_smod(1)
    emit_out_stage(0, 0, 2)
    emit_out_stage(0, 0, 3)
    emit_rank_mms("1a", dataT_h[0], 256, 0, 8)
    emit_rank_mms("1a", dataT_h[0], 256, 8, 16)
    emit_mul("1a", 1, 256, off=0)
    emit_out_stage("1a", BT, 0, split_store=True)
    emit_rank_mms("1b", dataT_h[1], 256, 0, 8)
    emit_out_stage("1a", BT, 1, split_store=True)
    emit_rank_mms("1b", dataT_h[1], 256, 8, 16)
    emit_mul("1b", 1, 256, off=256)
    emit_out_stage("1b", BT + 256, 0, split_store=True)
    emit_out_stage("1b", BT + 256, 1, fine_stores=True)


_CACHE = {}


def build():
    if "nc" in _CACHE:
        return _CACHE["nc"]
    nc = bacc.Bacc("TRN2", target_bir_lowering=False, debug=False)
    with tile.TileContext(nc) as tc, ExitStack() as ctx:
        _emit(nc, tc, ctx)
    nc.compile()
    _CACHE["nc"] = nc
    return nc


def make_in_maps(data, context, u, s, v, w, bias):
    u = np.ascontiguousarray(np.asarray(u, dtype=np.float32))
    s = np.ascontiguousarray(np.asarray(s, dtype=np.float32))
    vT = np.ascontiguousarray(np.asarray(v, dtype=np.float32).T)
    w = np.ascontiguousarray(np.asarray(w, dtype=np.float32))
    bias = np.ascontiguousarray(np.asarray(bias, dtype=np.float32))
    in_maps = []
    for c in range(NCORES):
        sl = slice(c * NB, (c + 1) * NB)
        in_maps.append(
            {
                "dataT": np.ascontiguousarray(np.asarray(data[sl], dtype=np.float32).T),
                "ctxT": np.ascontiguousarray(
                    np.asarray(context[sl], dtype=np.float32).T
                ),
                "u": u,
                "s": s,
                "vT": vT,
                "w": w,
                "bias": bias,
            }
        )
    return in_maps


def kernel(data, context, u, s, v, w, bias):
    nc = build()
    in_maps = make_in_maps(data, context, u, s, v, w, bias)
    res = run_bass_kernel_spmd(nc, in_maps, core_ids=list(range(NCORES)))
    return np.concatenate([r["out"] for r in res.results], axis=0)



# revision 6
# speedup vs baseline: 1.4278x; 1.4278x over previous
"""Trainium2 Bass kernel for nn_CADense (context-adaptive low-rank dense layer).

Computes, for the full batch:
    s_mod = s + context @ w          # [B, R]
    low   = (data @ u) * s_mod       # [B, R]
    out   = relu(low @ v.T + 2*bias) # [B, UNITS]

Sharding: data-parallel over batch across 8 NeuronCores; u/s/v/w/bias
replicated. Each core runs the same Bass program on its 1024-row shard.

The kernel is HBM-traffic bound at fp32 (23.6 MB/core ≈ 60+ us at per-core
HBM bandwidth), so all matmul operands are marshaled to bf16 on the host
and the output is stored bf16 (widened to fp32 host-side): 11.8 MB/core.
All accumulation stays fp32 in PSUM; measured end-to-end rel err ~1e-3
vs the 2e-2 gate. bias is all-zeros per the spec; a nonzero bias falls
back to an exact host computation.

Host-side marshaling lays every tensor out exactly as its SBUF tile
([128, ...] partition-major, contraction-dim-major free layout), so each
DMA is one contiguous segment per partition — fat descriptors, cheap
HWDGE dispatch. Loads are split across both HWDGE rings (sync: u-head +
data stream in consumption order; scalar: weights/context), stores ride
the gpsimd SWDGE ring, with the final store split across two rings to
shorten the tail.

Compute per 512-row batch tile, in the transposed domain per rank-chunk:
    pd[r, b]  = (u.T @ data.T)[r, b]          (16 K-chunks into PSUM)
    ps[r, b]  = (w.T @ ctx.T)[r, b]           (4 K-chunks into PSUM)
    lowT[r,b] = (ps[r,b] + s[r]) * pd[r,b]    (one fused DVE op, bf16 out)
    out[b, m] = relu(lowT.T @ v.T)            (PSUM; scalar/vector ReLU evac)
The batch-tile-1 rank stage is interleaved with batch-tile-0's output
stage so the PE never idles long enough for the HAM clock gate to
re-throttle; a short burst of dummy bf16 matmuls pre-warms the gate
while the first DMAs stream in.
"""

import os
import sys
from contextlib import ExitStack

import numpy as np

try:
    import ml_dtypes  # noqa: F401

    BF16_NP = np.dtype("bfloat16")
except (ImportError, TypeError):
    from jax import numpy as _jnp  # pragma: no cover

    BF16_NP = _jnp.bfloat16


def _ensure_concourse():
    try:
        import concourse  # noqa: F401
    except ImportError:
        for p in ("/opt/trn_rl_repo", "/root/.axon_site/_ro/trn_rl_repo"):
            if os.path.isdir(p) and p not in sys.path:
                sys.path.insert(0, p)


_ensure_concourse()

import concourse.tile as tile  # noqa: E402
from concourse import bacc, mybir  # noqa: E402
from concourse.bass_utils import run_bass_kernel_spmd  # noqa: E402

NCORES = 8
B, N_IN, UNITS, RANK, CCTX = 8192, 2048, 2048, 256, 512
NB = B // NCORES  # batch rows per core
P = 128
BT = 512  # batch tile (free dim of T-domain matmuls)
NBT = NB // BT  # batch tiles per core
KC = N_IN // P  # 16 contraction chunks for data @ u
CC = CCTX // P  # 4 contraction chunks for context @ w
RC = RANK // P  # 2 rank chunks
MS = 512  # output units slice width
NMS = UNITS // MS  # 4 unit slices
# data DMA chunking (in KC units) per batch tile: small head chunks so the
# first rank matmuls start early, bigger ones once the pipe is primed.
GROUPS0 = (2, 2, 4, 4, 4)
GROUPS1 = (8, 8)
U_HEAD = 4  # leading u K-chunks loaded on the sync ring ahead of the data
N_WARMUP_MM = 16
WU_N = 256  # warmup matmul free dim

F32 = mybir.dt.float32
BF16 = mybir.dt.bfloat16
RELU = mybir.ActivationFunctionType.Relu
ADD = mybir.AluOpType.add
MULT = mybir.AluOpType.mult


def _emit(nc, tc, ctx):
    d_dataT = nc.dram_tensor("dataT", [P, NBT, KC, BT], BF16, kind="ExternalInput")
    d_ctxT = nc.dram_tensor("ctxT", [P, NBT, CC, BT], BF16, kind="ExternalInput")
    d_u = nc.dram_tensor("u", [P, KC, RANK], BF16, kind="ExternalInput")
    d_s = nc.dram_tensor("s", [P, RC], F32, kind="ExternalInput")
    d_vT = nc.dram_tensor("vT", [P, RC, UNITS], BF16, kind="ExternalInput")
    d_w = nc.dram_tensor("w", [P, CC, RANK], BF16, kind="ExternalInput")
    d_out = nc.dram_tensor("out", [NB, UNITS], BF16, kind="ExternalOutput")

    singles = ctx.enter_context(tc.tile_pool(name="singles", bufs=1))
    du_psum = ctx.enter_context(tc.tile_pool(name="du_psum", bufs=2, space="PSUM"))
    s_psum = ctx.enter_context(tc.tile_pool(name="s_psum", bufs=2, space="PSUM"))
    o_psum = ctx.enter_context(tc.tile_pool(name="o_psum", bufs=4, space="PSUM"))

    # ---- SBUF tiles (all single-use: no pool-recycle stalls on DMA rings) --
    u_sb = singles.tile([P, KC, RANK], BF16)
    w_sb = singles.tile([P, CC, RANK], BF16)
    s_sb = singles.tile([P, RC], F32)
    vT_sb = singles.tile([P, RC, UNITS], BF16)
    ctx_sb = [singles.tile([P, CC, BT], BF16, name=f"ctx{bt}") for bt in range(NBT)]
    smod = [
        [singles.tile([P, BT], F32, name=f"smod{bt}r{rc}") for rc in range(RC)]
        for bt in range(NBT)
    ]
    dt = {}
    for bt, groups in ((0, GROUPS0), (1, GROUPS1)):
        kc0 = 0
        for gi, g in enumerate(groups):
            dt[(bt, gi)] = singles.tile([P, g, BT], BF16, name=f"dt{bt}g{gi}")
            kc0 += g
    lowT = [singles.tile([P, RC, BT], BF16, name=f"lowT{bt}") for bt in range(NBT)]
    osb = [singles.tile([P, UNITS], BF16, name=f"osb{i}") for i in range(NBT * 4)]
    wu_a = singles.tile([P, P], BF16)
    wu_b = singles.tile([P, WU_N], BF16)

    # ---- DMA dispatch, emitted first so both HWDGE rings start at t=0 ----
    # sync ring: u head + the data stream, in consumption order.
    nc.sync.dma_start(out=u_sb[:, 0:U_HEAD], in_=d_u.ap()[:, 0:U_HEAD])
    kc0 = 0
    for gi, g in enumerate(GROUPS0):
        nc.sync.dma_start(out=dt[(0, gi)][:], in_=d_dataT.ap()[:, 0, kc0 : kc0 + g])
        kc0 += g
    kc0 = 0
    for gi, g in enumerate(GROUPS1):
        nc.sync.dma_start(out=dt[(1, gi)][:], in_=d_dataT.ap()[:, 1, kc0 : kc0 + g])
        kc0 += g
    # scalar ring: the rest of the (small) operands, in consumption order.
    nc.scalar.dma_start(out=u_sb[:, U_HEAD:], in_=d_u.ap()[:, U_HEAD:])
    nc.scalar.dma_start(out=w_sb[:], in_=d_w.ap())
    nc.scalar.dma_start(out=ctx_sb[0][:], in_=d_ctxT.ap()[:, 0])
    nc.scalar.dma_start(out=s_sb[:], in_=d_s.ap())
    nc.scalar.dma_start(out=vT_sb[:], in_=d_vT.ap())
    nc.scalar.dma_start(out=ctx_sb[1][:], in_=d_ctxT.ap()[:, 1])

    # ---- HAM warm-up: dummy bf16 matmuls while the first loads stream ----
    nc.vector.memset(wu_a[:], 1.0)
    nc.vector.memset(wu_b[:], 1.0)
    wu_ps = o_psum.tile([P, MS], F32, tag="po", name="wu_ps")
    for _ in range(N_WARMUP_MM):
        nc.tensor.matmul(
            wu_ps[:, 0:WU_N], lhsT=wu_a[:], rhs=wu_b[:], start=True, stop=True
        )

    # ---- compute stages ------------------------------------------------
    pd = {}
    ps = {}

    def emit_rank_group(bt, gi, kc0, g):
        """(u.T @ dataT) accumulation for one data chunk, both rank halves."""
        if kc0 == 0:
            pd[bt] = [
                du_psum.tile([P, BT], F32, tag="pd", name="pd") for _ in range(RC)
            ]
        for kc in range(kc0, kc0 + g):
            for rc in range(RC):
                nc.tensor.matmul(
                    pd[bt][rc][:],
                    lhsT=u_sb[:, kc, rc * P : (rc + 1) * P],
                    rhs=dt[(bt, gi)][:, kc - kc0, :],
                    start=(kc == 0),
                    stop=(kc == KC - 1),
                )

    def emit_smod(bt):
        """(w.T @ ctxT) accumulation; independent of the data stream."""
        ps[bt] = [s_psum.tile([P, BT], F32, tag="ps", name="ps") for _ in range(RC)]
        for rc in range(RC):
            for cc in range(CC):
                nc.tensor.matmul(
                    ps[bt][rc][:],
                    lhsT=w_sb[:, cc, rc * P : (rc + 1) * P],
                    rhs=ctx_sb[bt][:, cc, :],
                    start=(cc == 0),
                    stop=(cc == CC - 1),
                )

    def emit_smod_evac(bt):
        """smod = ps + s on the scalar engine (PSUM -> SBUF)."""
        for rc in range(RC):
            nc.scalar.add(smod[bt][rc][:], ps[bt][rc][:], add=s_sb[:, rc : rc + 1])

    def emit_low(bt):
        """lowT = pd * smod on the vector engine, bf16 out."""
        for rc in range(RC):
            nc.vector.tensor_mul(
                out=lowT[bt][:, rc, :], in0=pd[bt][rc][:], in1=smod[bt][rc][:]
            )

    def emit_out_stage(bt, bc, split_last=False):
        """out[rows, :] = relu(lowT.T @ vT) for one 128-row chunk + store."""
        pos = [o_psum.tile([P, MS], F32, tag="po", name="po") for _ in range(NMS)]
        for rc in range(RC):
            for ms in range(NMS):
                nc.tensor.matmul(
                    pos[ms][:],
                    lhsT=lowT[bt][:, rc, bc * P : (bc + 1) * P],
                    rhs=vT_sb[:, rc, ms * MS : (ms + 1) * MS],
                    start=(rc == 0),
                    stop=(rc == RC - 1),
                )
        o = osb[bt * 4 + bc]
        for ms in range(NMS):
            sl = slice(ms * MS, (ms + 1) * MS)
            if ms % 2 == 0:
                nc.scalar.activation(o[:, sl], pos[ms][:], RELU)
            else:
                nc.vector.tensor_relu(out=o[:, sl], in_=pos[ms][:])
        r0 = bt * BT + bc * P
        rows = slice(r0, r0 + P)
        if split_last:
            h = UNITS // 2
            nc.gpsimd.dma_start(out=d_out.ap()[rows, :h], in_=o[:, :h])
            nc.scalar.dma_start(out=d_out.ap()[rows, h:], in_=o[:, h:])
        else:
            nc.gpsimd.dma_start(out=d_out.ap()[rows, :], in_=o[:])

    # Software pipeline: PE emission ordered to match DMA arrival order;
    # bt1's rank stage fills the gaps in bt0's output stage.
    emit_rank_group(0, 0, 0, GROUPS0[0])
    emit_rank_group(0, 1, 2, GROUPS0[1])
    emit_smod(0)
    emit_smod_evac(0)
    emit_rank_group(0, 2, 4, GROUPS0[2])
    emit_rank_group(0, 3, 8, GROUPS0[3])
    emit_rank_group(0, 4, 12, GROUPS0[4])
    emit_low(0)
    emit_out_stage(0, 0)
    emit_out_stage(0, 1)
    emit_rank_group(1, 0, 0, GROUPS1[0])
    emit_out_stage(0, 2)
    emit_smod(1)
    emit_smod_evac(1)
    emit_rank_group(1, 1, 8, GROUPS1[1])
    emit_out_stage(0, 3)
    emit_low(1)
    emit_out_stage(1, 0)
    emit_out_stage(1, 1)
    emit_out_stage(1, 2)
    emit_out_stage(1, 3, split_last=True)


_CACHE = {}


def build():
    if "nc" in _CACHE:
        return _CACHE["nc"]
    nc = bacc.Bacc("TRN2", target_bir_lowering=False, debug=False)
    with tile.TileContext(nc) as tc, ExitStack() as ctx:
        _emit(nc, tc, ctx)
    nc.compile()
    _CACHE["nc"] = nc
    return nc


def make_in_maps(data, context, u, s, v, w, bias):
    data16 = np.asarray(data, dtype=np.float32).astype(BF16_NP)
    ctx16 = np.asarray(context, dtype=np.float32).astype(BF16_NP)
    u16 = np.ascontiguousarray(
        np.asarray(u, dtype=np.float32).astype(BF16_NP).reshape(KC, P, RANK)
        .transpose(1, 0, 2)
    )
    w16 = np.ascontiguousarray(
        np.asarray(w, dtype=np.float32).astype(BF16_NP).reshape(CC, P, RANK)
        .transpose(1, 0, 2)
    )
    vT16 = np.ascontiguousarray(
        np.asarray(v, dtype=np.float32).astype(BF16_NP).T.reshape(RC, P, UNITS)
        .transpose(1, 0, 2)
    )
    s32 = np.ascontiguousarray(np.asarray(s, dtype=np.float32).reshape(RC, P).T)
    in_maps = []
    for c in range(NCORES):
        sl = slice(c * NB, (c + 1) * NB)
        in_maps.append(
            {
                "dataT": np.ascontiguousarray(
                    data16[sl].reshape(NBT, BT, KC, P).transpose(3, 0, 2, 1)
                ),
                "ctxT": np.ascontiguousarray(
                    ctx16[sl].reshape(NBT, BT, CC, P).transpose(3, 0, 2, 1)
                ),
                "u": u16,
                "s": s32,
                "vT": vT16,
                "w": w16,
            }
        )
    return in_maps


def kernel(data, context, u, s, v, w, bias):
    bias = np.asarray(bias, dtype=np.float32)
    if np.any(bias):
        # Reference path (bias is all-zeros per the problem spec; keep the
        # general case exact rather than specializing the device kernel).
        data = np.asarray(data, dtype=np.float32)
        context = np.asarray(context, dtype=np.float32)
        u = np.asarray(u, dtype=np.float32)
        s = np.asarray(s, dtype=np.float32)
        v = np.asarray(v, dtype=np.float32)
        w = np.asarray(w, dtype=np.float32)
        s_mod = s + context @ w
        low = (data @ u) * s_mod
        out = low @ v.T + 2.0 * bias
        return np.maximum(out, 0.0).astype(np.float32)
    nc = build()
    in_maps = make_in_maps(data, context, u, s, v, w, bias)
    res = run_bass_kernel_spmd(nc, in_maps, core_ids=list(range(NCORES)))
    return np.concatenate(
        [np.asarray(r["out"], dtype=np.float32) for r in res.results], axis=0
    )


# revision 12
# speedup vs baseline: 1.5062x; 1.0549x over previous
"""Trainium2 Bass kernel for nn_CADense (context-adaptive low-rank dense layer).

Computes, for the full batch:
    s_mod = s + context @ w          # [B, R]
    low   = (data @ u) * s_mod       # [B, R]
    out   = relu(low @ v.T + 2*bias) # [B, UNITS]

Sharding: data-parallel over batch across 8 NeuronCores; u/s/v/w/bias
replicated. Each core runs the same Bass program on its 1024-row shard.

The kernel is HBM-traffic bound at fp32 (23.6 MB/core ≈ 60+ us at per-core
HBM bandwidth), so all matmul operands are marshaled to bf16 on the host
and the output is stored bf16 (widened to fp32 host-side): 11.8 MB/core.
All accumulation stays fp32 in PSUM; measured end-to-end rel err ~1e-3
vs the 2e-2 gate. bias is all-zeros per the spec; a nonzero bias falls
back to an exact host computation.

Host-side marshaling lays every tensor out exactly as its SBUF tile
([128, ...] partition-major, contraction-dim-major free layout), so each
DMA is one contiguous segment per partition — fat descriptors, cheap
HWDGE dispatch. Loads are split across both HWDGE rings (sync: u-head +
data stream in consumption order; scalar: weights/context), stores ride
the gpsimd SWDGE ring, with the final store split across two rings to
shorten the tail.

Compute per 512-row batch tile, in the transposed domain per rank-chunk:
    pd[r, b]  = (u.T @ data.T)[r, b]          (16 K-chunks into PSUM)
    ps[r, b]  = (w.T @ ctx.T)[r, b]           (4 K-chunks into PSUM)
    lowT[r,b] = (ps[r,b] + s[r]) * pd[r,b]    (one fused DVE op, bf16 out)
    out[b, m] = relu(lowT.T @ v.T)            (PSUM; scalar/vector ReLU evac)
The batch-tile-1 rank stage is interleaved with batch-tile-0's output
stage so the PE never idles long enough for the HAM clock gate to
re-throttle; a short burst of dummy bf16 matmuls pre-warms the gate
while the first DMAs stream in.
"""

import os
import sys
from contextlib import ExitStack

import numpy as np

try:
    import ml_dtypes  # noqa: F401

    BF16_NP = np.dtype("bfloat16")
except (ImportError, TypeError):
    from jax import numpy as _jnp  # pragma: no cover

    BF16_NP = _jnp.bfloat16


def _ensure_concourse():
    try:
        import concourse  # noqa: F401
    except ImportError:
        for p in ("/opt/trn_rl_repo", "/root/.axon_site/_ro/trn_rl_repo"):
            if os.path.isdir(p) and p not in sys.path:
                sys.path.insert(0, p)


_ensure_concourse()

import concourse.tile as tile  # noqa: E402
from concourse import bacc, mybir  # noqa: E402
from concourse.bass_utils import run_bass_kernel_spmd  # noqa: E402

NCORES = 8
B, N_IN, UNITS, RANK, CCTX = 8192, 2048, 2048, 256, 512
NB = B // NCORES  # batch rows per core
P = 128
BT = 512  # batch tile (free dim of T-domain matmuls)
NBT = NB // BT  # batch tiles per core
KC = N_IN // P  # 16 contraction chunks for data @ u
CC = CCTX // P  # 4 contraction chunks for context @ w
RC = RANK // P  # 2 rank chunks
MS = 512  # output units slice width
NMS = UNITS // MS  # 4 unit slices
# data DMA chunking (in KC units) per batch tile: small head chunks so the
# first rank matmuls start early, bigger ones once the pipe is primed.
GROUPS0 = (2, 2, 4, 4, 4)
GROUPS1 = (8, 8)
N_WARMUP_MM = 8
WU_N = 256  # warmup matmul free dim

F32 = mybir.dt.float32
BF16 = mybir.dt.bfloat16
RELU = mybir.ActivationFunctionType.Relu
ADD = mybir.AluOpType.add
MULT = mybir.AluOpType.mult


def _emit(nc, tc, ctx):
    d_dataT = nc.dram_tensor("dataT", [P, NBT, KC, BT], BF16, kind="ExternalInput")
    d_ctxT = nc.dram_tensor("ctxT", [P, NBT, CC, BT], BF16, kind="ExternalInput")
    d_u = nc.dram_tensor("u", [P, KC, RANK], BF16, kind="ExternalInput")
    d_s = nc.dram_tensor("s", [P, RC], F32, kind="ExternalInput")
    d_vT = nc.dram_tensor("vT", [P, RC, UNITS], BF16, kind="ExternalInput")
    d_w = nc.dram_tensor("w", [P, CC, RANK], BF16, kind="ExternalInput")
    d_out = nc.dram_tensor("out", [NB, UNITS], BF16, kind="ExternalOutput")

    singles = ctx.enter_context(tc.tile_pool(name="singles", bufs=1))
    du_psum = ctx.enter_context(tc.tile_pool(name="du_psum", bufs=2, space="PSUM"))
    s_psum = ctx.enter_context(tc.tile_pool(name="s_psum", bufs=1, space="PSUM"))
    o_psum = ctx.enter_context(tc.tile_pool(name="o_psum", bufs=5, space="PSUM"))

    # ---- SBUF tiles (all single-use: no pool-recycle stalls on DMA rings) --
    u_sb = singles.tile([P, KC, RANK], BF16)
    w_sb = singles.tile([P, CC, RANK], BF16)
    s_sb = singles.tile([P, RC], F32)
    vT_sb = singles.tile([P, RC, UNITS], BF16)
    ctx_sb = [singles.tile([P, CC, BT], BF16, name=f"ctx{bt}") for bt in range(NBT)]
    smod = [
        [singles.tile([P, BT], F32, name=f"smod{bt}r{rc}") for rc in range(RC)]
        for bt in range(NBT)
    ]
    dt = {}
    for bt, groups in ((0, GROUPS0), (1, GROUPS1)):
        kc0 = 0
        for gi, g in enumerate(groups):
            dt[(bt, gi)] = singles.tile([P, g, BT], BF16, name=f"dt{bt}g{gi}")
            kc0 += g
    lowT = [singles.tile([P, RC, BT], BF16, name=f"lowT{bt}") for bt in range(NBT)]
    osb = [singles.tile([P, UNITS], BF16, name=f"osb{i}") for i in range(NBT * 4)]
    wu_a = singles.tile([P, P], BF16)
    wu_b = singles.tile([P, WU_N], BF16)

    # ---- DMA dispatch, emitted first so both HWDGE rings start at t=0 ----
    # The bt0-critical prefix (u, data bt0, w, ctx0, vT) is balanced
    # byte-for-byte across the two HWDGE rings in consumption order so the
    # aggregate HBM bandwidth all goes to what the PE needs next; bt1 data
    # and ctx1 queue behind it. Stores ride gpsimd (see emit_out_stage).
    nc.sync.dma_start(out=u_sb[:, 0:2], in_=d_u.ap()[:, 0:2])
    nc.sync.dma_start(out=dt[(0, 0)][:], in_=d_dataT.ap()[:, 0, 0:2])
    nc.sync.dma_start(out=dt[(0, 2)][:], in_=d_dataT.ap()[:, 0, 4:8])
    nc.sync.dma_start(out=ctx_sb[0][:], in_=d_ctxT.ap()[:, 0])
    nc.sync.dma_start(out=dt[(0, 4)][:], in_=d_dataT.ap()[:, 0, 12:16])
    nc.sync.dma_start(out=vT_sb[:, :, 0:1024], in_=d_vT.ap()[:, :, 0:1024])
    nc.sync.dma_start(out=dt[(1, 0)][:], in_=d_dataT.ap()[:, 1, 0:8])

    nc.scalar.dma_start(out=u_sb[:, 2:4], in_=d_u.ap()[:, 2:4])
    nc.scalar.dma_start(out=dt[(0, 1)][:], in_=d_dataT.ap()[:, 0, 2:4])
    nc.scalar.dma_start(out=u_sb[:, 4:], in_=d_u.ap()[:, 4:])
    nc.scalar.dma_start(out=w_sb[:], in_=d_w.ap())
    nc.scalar.dma_start(out=s_sb[:], in_=d_s.ap())
    nc.scalar.dma_start(out=dt[(0, 3)][:], in_=d_dataT.ap()[:, 0, 8:12])
    nc.scalar.dma_start(out=vT_sb[:, :, 1024:], in_=d_vT.ap()[:, :, 1024:])
    nc.scalar.dma_start(out=dt[(1, 1)][:], in_=d_dataT.ap()[:, 1, 8:16])
    nc.scalar.dma_start(out=ctx_sb[1][:], in_=d_ctxT.ap()[:, 1])

    # ---- HAM warm-up: dummy bf16 matmuls while the first loads stream ----
    nc.vector.memset(wu_a[:], 1.0)
    nc.vector.memset(wu_b[:], 1.0)
    wu_ps = o_psum.tile([P, MS], F32, tag="po", name="wu_ps")
    for _ in range(N_WARMUP_MM):
        nc.tensor.matmul(
            wu_ps[:, 0:WU_N], lhsT=wu_a[:], rhs=wu_b[:], start=True, stop=True
        )

    # ---- compute stages ------------------------------------------------
    pd = {}
    ps = {}

    def emit_rank_group(bt, gi, kc0, g):
        """(u.T @ dataT) accumulation for one data chunk, both rank halves."""
        if kc0 == 0:
            pd[bt] = [
                du_psum.tile([P, BT], F32, tag="pd", name="pd") for _ in range(RC)
            ]
        for kc in range(kc0, kc0 + g):
            for rc in range(RC):
                nc.tensor.matmul(
                    pd[bt][rc][:],
                    lhsT=u_sb[:, kc, rc * P : (rc + 1) * P],
                    rhs=dt[(bt, gi)][:, kc - kc0, :],
                    start=(kc == 0),
                    stop=(kc == KC - 1),
                )

    def emit_smod_rc(bt, rc):
        """(w.T @ ctxT) accumulation for one rank half; the single-buf ps
        ring serializes rc1 behind rc0's evacuation automatically."""
        ps[(bt, rc)] = s_psum.tile([P, BT], F32, tag="ps", name="ps")
        for cc in range(CC):
            nc.tensor.matmul(
                ps[(bt, rc)][:],
                lhsT=w_sb[:, cc, rc * P : (rc + 1) * P],
                rhs=ctx_sb[bt][:, cc, :],
                start=(cc == 0),
                stop=(cc == CC - 1),
            )
        # smod = ps + s on the scalar engine (PSUM -> SBUF), freeing ps.
        nc.scalar.add(smod[bt][rc][:], ps[(bt, rc)][:], add=s_sb[:, rc : rc + 1])

    def emit_low(bt):
        """lowT = pd * smod on the vector engine, bf16 out."""
        for rc in range(RC):
            nc.vector.tensor_mul(
                out=lowT[bt][:, rc, :], in0=pd[bt][rc][:], in1=smod[bt][rc][:]
            )

    def emit_out_stage(bt, bc, store="gpsimd"):
        """out[rows, :] = relu(lowT.T @ vT) for one 128-row chunk + store."""
        pos = [o_psum.tile([P, MS], F32, tag="po", name="po") for _ in range(NMS)]
        for rc in range(RC):
            for ms in range(NMS):
                nc.tensor.matmul(
                    pos[ms][:],
                    lhsT=lowT[bt][:, rc, bc * P : (bc + 1) * P],
                    rhs=vT_sb[:, rc, ms * MS : (ms + 1) * MS],
                    start=(rc == 0),
                    stop=(rc == RC - 1),
                )
        o = osb[bt * 4 + bc]
        for ms in range(NMS):
            sl = slice(ms * MS, (ms + 1) * MS)
            if ms % 2 == 0:
                nc.scalar.activation(o[:, sl], pos[ms][:], RELU)
            else:
                nc.vector.tensor_relu(out=o[:, sl], in_=pos[ms][:])
        r0 = bt * BT + bc * P
        rows = slice(r0, r0 + P)
        if store == "split":
            h = UNITS // 2
            nc.sync.dma_start(out=d_out.ap()[rows, :h], in_=o[:, :h])
            nc.scalar.dma_start(out=d_out.ap()[rows, h:], in_=o[:, h:])
        elif store == "sync":
            nc.sync.dma_start(out=d_out.ap()[rows, :], in_=o[:])
        else:
            nc.gpsimd.dma_start(out=d_out.ap()[rows, :], in_=o[:])

    # Software pipeline: PE emission ordered to match DMA arrival order;
    # bt1's rank stage fills the gaps in bt0's output stage.
    emit_rank_group(0, 0, 0, GROUPS0[0])
    emit_rank_group(0, 1, 2, GROUPS0[1])
    emit_rank_group(0, 2, 4, GROUPS0[2])
    emit_rank_group(0, 3, 8, GROUPS0[3])
    emit_smod_rc(0, 0)
    emit_rank_group(0, 4, 12, GROUPS0[4])
    emit_smod_rc(0, 1)
    emit_low(0)
    emit_out_stage(0, 0)
    emit_out_stage(0, 1)
    emit_out_stage(0, 2)
    emit_rank_group(1, 0, 0, GROUPS1[0])
    emit_out_stage(0, 3)
    emit_smod_rc(1, 0)
    emit_rank_group(1, 1, 8, GROUPS1[1])
    emit_smod_rc(1, 1)
    emit_low(1)
    emit_out_stage(1, 0)
    emit_out_stage(1, 1)
    emit_out_stage(1, 2, store="sync")
    emit_out_stage(1, 3, store="split")


_CACHE = {}


def build():
    if "nc" in _CACHE:
        return _CACHE["nc"]
    nc = bacc.Bacc("TRN2", target_bir_lowering=False, debug=False)
    with tile.TileContext(nc) as tc, ExitStack() as ctx:
        _emit(nc, tc, ctx)
    nc.compile()
    _CACHE["nc"] = nc
    return nc


def make_in_maps(data, context, u, s, v, w, bias):
    data16 = np.asarray(data, dtype=np.float32).astype(BF16_NP)
    ctx16 = np.asarray(context, dtype=np.float32).astype(BF16_NP)
    u16 = np.ascontiguousarray(
        np.asarray(u, dtype=np.float32).astype(BF16_NP).reshape(KC, P, RANK)
        .transpose(1, 0, 2)
    )
    w16 = np.ascontiguousarray(
        np.asarray(w, dtype=np.float32).astype(BF16_NP).reshape(CC, P, RANK)
        .transpose(1, 0, 2)
    )
    vT16 = np.ascontiguousarray(
        np.asarray(v, dtype=np.float32).astype(BF16_NP).T.reshape(RC, P, UNITS)
        .transpose(1, 0, 2)
    )
    s32 = np.ascontiguousarray(np.asarray(s, dtype=np.float32).reshape(RC, P).T)
    in_maps = []
    for c in range(NCORES):
        sl = slice(c * NB, (c + 1) * NB)
        in_maps.append(
            {
                "dataT": np.ascontiguousarray(
                    data16[sl].reshape(NBT, BT, KC, P).transpose(3, 0, 2, 1)
                ),
                "ctxT": np.ascontiguousarray(
                    ctx16[sl].reshape(NBT, BT, CC, P).transpose(3, 0, 2, 1)
                ),
                "u": u16,
                "s": s32,
                "vT": vT16,
                "w": w16,
            }
        )
    return in_maps


def kernel(data, context, u, s, v, w, bias):
    bias = np.asarray(bias, dtype=np.float32)
    if np.any(bias):
        # Reference path (bias is all-zeros per the problem spec; keep the
        # general case exact rather than specializing the device kernel).
        data = np.asarray(data, dtype=np.float32)
        context = np.asarray(context, dtype=np.float32)
        u = np.asarray(u, dtype=np.float32)
        s = np.asarray(s, dtype=np.float32)
        v = np.asarray(v, dtype=np.float32)
        w = np.asarray(w, dtype=np.float32)
        s_mod = s + context @ w
        low = (data @ u) * s_mod
        out = low @ v.T + 2.0 * bias
        return np.maximum(out, 0.0).astype(np.float32)
    nc = build()
    in_maps = make_in_maps(data, context, u, s, v, w, bias)
    res = run_bass_kernel_spmd(nc, in_maps, core_ids=list(range(NCORES)))
    return np.concatenate(
        [np.asarray(r["out"], dtype=np.float32) for r in res.results], axis=0
    )
